# revision 1
# baseline (speedup 1.0000x reference)
"""Trainium2 Bass kernel for a 3-layer GCN (Kipf-Welling, symmetric norm,
self-loops) with global add pooling.

Distribution: nodes (graph-aligned contiguous ranges) are sharded across 8
NeuronCores.  Each core owns the aggregation (scatter-add) for its local dst
nodes; the per-layer activations are exchanged with an AllGather so every core
can gather arbitrary source rows with indirect DMA.

Math (matches the jax reference exactly):
    deg  = indeg + 1, dis = deg^-1/2
    Hs   = dis * (H @ W)              (rows scaled by dis)
    agg  = dis_dst * sum_{e:(s->d)} Hs[s]   over edges *including self-loops*
         = sum_e dis_s dis_d (HW)[s] + (HW)[d]/deg_d
    H'   = relu(agg + b)              (no relu on layer 3)
    out  = segment_sum(H3, batch)

Feature-major layout on chip: H^T tiles [128 feats, nodes] so the layer
matmul streams with W as the stationary operand.  The edge scatter-add is a
matmul with an on-the-fly selection matrix S[e, d] = (dst_id[e] == d), built
on the vector engine by comparing per-edge dst ids against an iota row.
"""

import os
import sys
import math

import numpy as np

sys.path.insert(0, "/opt/trn_rl_repo")

import concourse.bass as bass  # noqa: E402
import concourse.bacc as bacc  # noqa: E402
import concourse.tile as tile  # noqa: E402
from concourse import mybir  # noqa: E402
from concourse.bass_utils import run_bass_kernel_spmd  # noqa: E402
from concourse.masks import make_identity  # noqa: E402

P = 128
F32 = mybir.dt.float32
F16 = mybir.dt.float16
I32 = mybir.dt.int32
I16 = mybir.dt.int16
OP = mybir.AluOpType

N_CORES = 8
G_TOTAL = 1000  # graphs in the batch (fixed by the problem)


# ----------------------------------------------------------------------------
# Host-side preprocessing: shard nodes/edges, build gather/selection metadata.
# ----------------------------------------------------------------------------

def _preprocess(x, edge_index, batch, n_cores, G):
    N = x.shape[0]
    src = edge_index[0].astype(np.int64)
    dst = edge_index[1].astype(np.int64)
    batch = batch.astype(np.int64)

    # graph-aligned shard boundaries near equal node counts
    graph_start = np.searchsorted(batch, np.arange(G + 1))  # [G+1], node idx
    bounds = [0]
    for c in range(1, n_cores):
        target = (c * N) // n_cores
        gi = np.searchsorted(graph_start, target)
        lo = graph_start[gi - 1] if gi > 0 else 0
        hi = graph_start[gi] if gi <= G else N
        b = int(hi if (hi - target) <= (target - lo) else lo)
        b = max(b, bounds[-1])  # keep non-decreasing
        bounds.append(b)
    bounds.append(N)
    bounds = np.asarray(bounds, dtype=np.int64)

    shard_sizes = bounds[1:] - bounds[:-1]
    N_loc = int(math.ceil(int(shard_sizes.max()) / P) * P)
    T = N_loc // P

    # normalization (index-derived scalars)
    deg = np.bincount(dst, minlength=N).astype(np.float32) + np.float32(1.0)
    dis = (np.float32(1.0) / np.sqrt(deg)).astype(np.float32)

    # padded-global source row ids (rows of the allgathered Hs table)
    core_of = np.searchsorted(bounds, dst, side="right") - 1
    core_of_src = np.searchsorted(bounds, src, side="right") - 1
    src_pg = core_of_src * N_loc + (src - bounds[core_of_src])

    # src-table quarters: int16 gather indices must stay < 32768
    QC = 2                      # cores per quarter
    NQ = n_cores // QC          # quarters
    QR = QC * N_loc             # rows per quarter
    TB = 2                      # tiles per gather block
    assert QR <= 32767, (QR, N_loc)

    per_core = []
    CPS = 1
    for c in range(n_cores):
        n_real = int(bounds[c + 1] - bounds[c])
        m = core_of == c
        dstl = np.concatenate([dst[m] - bounds[c], np.arange(n_real)])
        srcs = np.concatenate([src_pg[m], c * N_loc + np.arange(n_real)])
        tile_id = dstl // P
        quarter = srcs // QR
        key = tile_id * NQ + quarter
        order = np.argsort(key, kind="stable")
        dstl, srcs, tile_id, quarter, key = (
            dstl[order], srcs[order], tile_id[order], quarter[order],
            key[order])
        counts = np.bincount(key, minlength=T * NQ)
        CPS = max(CPS, int(math.ceil(int(counts.max()) / P)))
        per_core.append((n_real, dstl, srcs, key, counts))

    NCH = T * NQ * CPS          # total chunks per core
    in_maps = []
    g_lo = []
    g_cnt = []
    GW = None
    for c in range(n_cores):
        n_real, dstl, srcs, key, counts = per_core[c]
        # slot grid: edge k of (tile,quarter) group -> chunk k//P, part k%P
        goff = np.concatenate([[0], np.cumsum(counts)])[:-1]
        rank = np.arange(dstl.shape[0]) - goff[key]
        chunk = rank // P
        part = rank % P
        col = key * CPS + chunk            # global chunk column (t, q, c)
        tile_id = key // NQ

        # int16 gather indices in gather-group order:
        # group (tile-block tb, quarter q) -> flat j = (t_loc*CPS+c)*128+p.
        # 16-partition-wrapped within each group, replicated x8 core groups.
        quarter = key % NQ
        tb = tile_id // TB
        t_loc = tile_id % TB
        tbg = np.minimum(TB, T - tb * TB)  # tiles in this block
        block_base = tb * NQ * TB * CPS    # chunk cols before this block
        gcol = block_base + quarter * tbg * CPS + t_loc * CPS + chunk
        flat = gcol * P + part
        idx16 = np.zeros((16, NCH * P // 16), dtype=np.int16)
        idx16[flat % 16, flat // 16] = (srcs % QR).astype(np.int16)
        srcidx = np.tile(idx16, (8, 1))
        dstid = np.full((P, NCH), 1.0e6, dtype=np.float32)
        dstid[part, col] = (dstl - tile_id * P).astype(np.float32)

        dis_loc = np.ones(N_loc, dtype=np.float32)
        dis_loc[:n_real] = dis[bounds[c]:bounds[c + 1]]
        disrep = np.broadcast_to(dis_loc.astype(np.float16), (P, N_loc)).copy()

        xT = np.zeros((P, N_loc), dtype=np.float32)
        xT[:, :n_real] = x[bounds[c]:bounds[c + 1]].T

        bloc = batch[bounds[c]:bounds[c + 1]]
        glo = int(bloc[0]) if n_real > 0 else 0
        gct = int(bloc[-1]) + 1 - glo if n_real > 0 else 0
        g_lo.append(glo)
        g_cnt.append(gct)
        in_maps.append(dict(srcidx=srcidx, dstid=dstid, disrep=disrep, xT=xT,
                            _bloc=bloc - glo, _n_real=n_real))

    GW = max(1, int(math.ceil(max(g_cnt) / P)))
    iota = np.broadcast_to(np.arange(P, dtype=np.float32), (P, P)).copy()
    for c in range(n_cores):
        d = in_maps[c]
        bloc, n_real = d.pop("_bloc"), d.pop("_n_real")
        poolid = np.full((P, T * GW), 1.0e6, dtype=np.float32)
        j = np.arange(n_real)
        for w in range(GW):
            poolid[j % P, (j // P) + w * T] = (bloc - w * P).astype(np.float32)
        d["poolid"] = poolid
        d["iota"] = iota

    cfg = dict(T=T, CPS=CPS, NQ=NQ, QR=QR, TB=TB, GW=GW, N_loc=N_loc,
               n_cores=n_cores)
    return cfg, in_maps, bounds, g_lo, g_cnt


# ----------------------------------------------------------------------------
# Bass program
# ----------------------------------------------------------------------------

def _build_program(cfg):
    T, CPS, GW, N_loc = cfg["T"], cfg["CPS"], cfg["GW"], cfg["N_loc"]
    NQ, QR, TB = cfg["NQ"], cfg["QR"], cfg["TB"]
    n_cores = cfg["n_cores"]
    NCH = T * NQ * CPS
    D, DO = 128, 64
    DOUT = {1: D, 2: D, 3: DO}

    nc = bacc.Bacc(None, num_devices=n_cores)

    xT_d = nc.dram_tensor("xT", [P, N_loc], F32, kind="ExternalInput")
    W_d = {0: nc.dram_tensor("W0", [D, D], F32, kind="ExternalInput"),
           1: nc.dram_tensor("W1", [D, D], F32, kind="ExternalInput"),
           2: nc.dram_tensor("W2", [D, D], F32, kind="ExternalInput"),
           3: nc.dram_tensor("W3", [D, DO], F32, kind="ExternalInput")}
    b_d = {l: nc.dram_tensor(f"b{l}", [P, 1], F32, kind="ExternalInput")
           for l in range(4)}
    srcidx_d = nc.dram_tensor("srcidx", [P, NCH * P // 16], I16,
                              kind="ExternalInput")
    dstid_d = nc.dram_tensor("dstid", [P, NCH], F32, kind="ExternalInput")
    disrep_d = nc.dram_tensor("disrep", [P, N_loc], F16, kind="ExternalInput")
    poolid_d = nc.dram_tensor("poolid", [P, T * GW], F32, kind="ExternalInput")
    iota_d = nc.dram_tensor("iota", [P, P], F32, kind="ExternalInput")
    out_d = nc.dram_tensor("out", [GW * P, DO], F32, kind="ExternalOutput")
    dbg_stage = os.environ.get("GCN_DBG_STAGE", "")
    dbg_d = dbg2_d = None
    if dbg_stage.startswith("h"):
        dbg_d = nc.dram_tensor("dbg", [P, N_loc], F32, kind="ExternalOutput")
    if dbg_stage.startswith("hsf"):
        dbg2_d = nc.dram_tensor("dbg2", [n_cores * N_loc, D], F32,
                                kind="ExternalOutput")

    with tile.TileContext(nc) as tc:
        with tc.tile_pool(name="const", bufs=1) as const, \
             tc.tile_pool(name="hpool", bufs=1) as hpool, \
             tc.tile_pool(name="stage", bufs=3) as stage, \
             tc.tile_pool(name="rpool", bufs=NQ + 1) as rpool, \
             tc.tile_pool(name="spool", bufs=1) as spool, \
             tc.tile_pool(name="dram", bufs=2, space="DRAM") as dram, \
             tc.tile_pool(name="pm", bufs=2, space="PSUM") as pm, \
             tc.tile_pool(name="pt", bufs=2, space="PSUM") as pt, \
             tc.tile_pool(name="pa", bufs=2, space="PSUM") as pa:

            # ---- constants into SBUF
            w_sb = {}
            for l in range(4):
                w = const.tile([D, DOUT.get(l, D) if l else D], F32,
                               name=f"w{l}sb")
                nc.sync.dma_start(out=w[:], in_=W_d[l][:, :])
                w_sb[l] = w
            b_sb = {}
            for l in range(4):
                b = const.tile([P, 1], F32, name=f"b{l}sb")
                nc.sync.dma_start(out=b[:], in_=b_d[l][:, :])
                b_sb[l] = b
            gmin = int(os.environ.get("GCN_MIN", "0"))
            iota_sb = const.tile([P, P], F32, name="iotasb")
            nc.sync.dma_start(out=iota_sb[:], in_=iota_d[:, :])
            iden_sb = const.tile([P, P], F32, name="idensb")
            if not gmin:
                make_identity(nc, iden_sb[:])
            srcidx_sb = const.tile([P, NCH * P // 16], I16, name="srcidxsb")
            nc.sync.dma_start(out=srcidx_sb[:], in_=srcidx_d[:, :])
            dstid_sb = const.tile([P, NCH], F32, name="dstidsb")
            disrep_sb = const.tile([P, N_loc], F16, name="disrepsb")
            poolid_sb = const.tile([P, T * GW], F32, name="poolidsb")
            if not gmin:
                nc.sync.dma_start(out=dstid_sb[:], in_=dstid_d[:, :])
                nc.sync.dma_start(out=disrep_sb[:], in_=disrep_d[:, :])
                nc.sync.dma_start(out=poolid_sb[:], in_=poolid_d[:, :])

            H = hpool.tile([P, N_loc], F32, name="H", tag="ha")

            # Join all const-load DMA sems into the DVE engine clock so later
            # DVE tensor_tensor ops (2 sync-wait slots in the ISA) don't have
            # to carry per-DMA waits themselves.
            if not gmin:
                joiner = const.tile([P, 1], F32, name="joiner")
                for cst in [iota_sb, dstid_sb, disrep_sb, poolid_sb,
                            b_sb[0], b_sb[1], b_sb[2], b_sb[3]]:
                    nc.vector.tensor_copy(out=joiner[:, :1], in_=cst[:, :1])

            # ---- phase 1: M^T = W^T @ Hprev^T, scale by dis, transpose,
            #      write node-major Hs to local DRAM (skipped for l=0/emb).
            def phase1(l, HprevT, HsLocal):
                dout = DOUT.get(l, D)
                nk = (N_loc + 511) // 512
                for k in range(nk):
                    c0 = k * 512
                    cw = min(512, N_loc - c0)
                    if l == 0:
                        # stream x^T chunks from DRAM (saves 6.5MB SBUF)
                        xst = stage.tile([P, 512], F32, name="xst", tag="ms")
                        nc.sync.dma_start(out=xst[:, :cw],
                                          in_=xT_d[:, c0:c0 + cw])
                        rhs_ap = xst[:, :cw]
                    else:
                        rhs_ap = HprevT[:, c0:c0 + cw]
                    mm = pm.tile([P, 512], F32, name="mm", tag="pm")
                    nc.tensor.matmul(mm[:dout, :cw], lhsT=w_sb[l][:, :dout],
                                     rhs=rhs_ap,
                                     start=True, stop=True)
                    if l == 0:
                        nc.vector.tensor_scalar(
                            out=H[:, c0:c0 + cw], in0=mm[:, :cw],
                            scalar1=b_sb[0][:, :], scalar2=None, op0=OP.add)
                        continue
                    ms = stage.tile([P, 512], F32, name="ms", tag="ms")
                    nc.vector.tensor_tensor(
                        out=ms[:dout, :cw], in0=mm[:dout, :cw],
                        in1=disrep_sb[:dout, c0:c0 + cw], op=OP.mult)
                    for tt in range(cw // P):
                        tglob = k * 4 + tt
                        ptt = pt.tile([P, P], F32, name="ptt", tag="pt")
                        nc.tensor.transpose(
                            out=ptt[:, :dout],
                            in_=ms[:dout, tt * P:(tt + 1) * P],
                            identity=iden_sb[:dout, :dout])
                        hs = stage.tile([P, P], F32, name="hs", tag="hs")
                        nc.vector.tensor_copy(out=hs[:, :dout],
                                              in_=ptt[:, :dout])
                        nc.sync.dma_start(
                            out=HsLocal[tglob * P:(tglob + 1) * P, :],
                            in_=hs[:, :dout])

            # ---- phase 2: gather + scatter-add into local dst tiles.
            def phase2(l, HsFull, H3):
                dout = DOUT[l]
                NB = (T + TB - 1) // TB
                for tb in range(NB):
                    tbg = min(TB, T - tb * TB)
                    block_base = tb * NQ * TB * CPS
                    num = tbg * CPS * P
                    Rq = []
                    for q in range(NQ):
                        R = rpool.tile([P, TB * CPS * dout], F32,
                                       name="R", tag="R")
                        c0 = (block_base + q * tbg * CPS) * P // 16
                        gi = phase2.gcount = getattr(phase2, "gcount", 0) + 1
                        real_n = int(os.environ.get("GCN_REAL_GATHERS",
                                                    "99999"))
                        if (int(os.environ.get("GCN_FAKE_GATHER", "0"))
                                or gi > real_n):
                            nc.sync.dma_start(
                                out=R[:, :tbg * CPS * dout],
                                in_=HsFull[q * QR:
                                           q * QR + P * tbg * CPS,
                                           :].rearrange(
                                    "(p c) e -> p (c e)", p=P))
                        else:
                            nc.gpsimd.dma_gather(
                                out_ap=R[:, :tbg * CPS * dout].rearrange(
                                    "p (c e) -> p c e", e=dout),
                                in_ap=HsFull[q * QR:(q + 1) * QR, :],
                                idxs_ap=srcidx_sb[:, c0:c0 + num // 16],
                                num_idxs=num,
                                num_idxs_reg=num,
                                elem_size=dout,
                                single_packet=(num <= 1008))
                        Rq.append(R)
                    if int(os.environ.get("GCN_NO_AGG", "0")):
                        for q in range(NQ):
                            nc.vector.tensor_copy(
                                out=H[:, (tb * NQ + q) % N_loc:
                                      (tb * NQ + q) % N_loc + 1],
                                in_=Rq[q][:, 0:1])
                        continue
                    for tl in range(tbg):
                        phase2_tile(l, tb * TB + tl, tl, Rq, H3)

            def phase2_tile(l, t, tl, Rq, H3):
                dout = DOUT[l]
                nch = NQ * CPS
                S = spool.tile([P, NQ * CPS * P], F32, name="S", tag="S")
                nc.vector.tensor_tensor(
                    out=S[:].rearrange("p (c d) -> p c d", d=P),
                    in0=dstid_sb[:, t * nch:(t + 1) * nch]
                        .unsqueeze(2).broadcast_to([P, nch, P]),
                    in1=iota_sb[:, :].unsqueeze(1).broadcast_to([P, nch, P]),
                    op=OP.is_equal)
                agg = pa.tile([P, P], F32, name="agg", tag="pa")
                for q in range(NQ):
                    for c in range(CPS):
                        k = q * CPS + c
                        nc.tensor.matmul(
                            agg[:dout, :],
                            lhsT=Rq[q][:, (tl * CPS + c) * dout:
                                       (tl * CPS + c + 1) * dout],
                            rhs=S[:, k * P:(k + 1) * P],
                            start=(k == 0), stop=(k == nch - 1))
                tmp = stage.tile([P, P], F32, name="tmp", tag="tmp")
                nc.vector.tensor_tensor(
                    out=tmp[:dout, :], in0=agg[:dout, :],
                    in1=disrep_sb[:dout, t * P:(t + 1) * P], op=OP.mult)
                if l < 3:
                    nc.vector.tensor_scalar(
                        out=H[:, t * P:(t + 1) * P], in0=tmp[:, :],
                        scalar1=b_sb[l][:, :], scalar2=0.0,
                        op0=OP.add, op1=OP.max)
                else:
                    t2 = stage.tile([P, P], F32, name="t2", tag="tmp")
                    nc.vector.tensor_scalar(
                        out=t2[:dout, :], in0=tmp[:dout, :],
                        scalar1=b_sb[3][:dout, :], scalar2=None,
                        op0=OP.add)
                    ptt = pt.tile([P, P], F32, name="ptt2", tag="pt")
                    nc.tensor.transpose(
                        out=ptt[:, :dout], in_=t2[:dout, :],
                        identity=iden_sb[:dout, :dout])
                    nc.vector.tensor_copy(
                        out=H3[:, t * DO:(t + 1) * DO],
                        in_=ptt[:, :dout])

            def dump_dbg(buf, width=None):
                nc.sync.dma_start(out=dbg_d[:, :width] if width else dbg_d[:, :],
                                  in_=buf[:, :width] if width else buf[:, :])

            # ---- the network
            max_layers = int(os.environ.get("GCN_MAX_LAYERS", "3"))
            skip_p1 = int(os.environ.get("GCN_SKIP_P1", "0"))
            if not skip_p1:
                phase1(0, None, None)  # embedding -> H (streams xT from DRAM)
            if dbg_stage == "h0":
                dump_dbg(H)
            H3 = None
            for l in range(1, max_layers + 1):
                dout = DOUT[l]
                HsLocal = dram.tile([N_loc, dout], F32, name=f"hsl{l}",
                                    tag="hsl")
                if skip_p1:
                    for r in range(T):
                        hz = stage.tile([P, P], F32, name="hz", tag="hs")
                        nc.vector.memset(hz[:, :dout], 1.0)
                        nc.sync.dma_start(
                            out=HsLocal[r * P:(r + 1) * P, :],
                            in_=hz[:, :dout])
                else:
                    phase1(l, H, HsLocal)
                HsFull = dram.tile(
                    [n_cores * N_loc, dout], F32, name=f"hsf{l}", tag="hsf",
                    addr_space="Local"
                    if int(os.environ.get("GCN_LOCAL_HSF", "0"))
                    else "Shared")
                if n_cores > 1 and not int(os.environ.get("GCN_NO_CC", "0")):
                    nc.gpsimd.collective_compute(
                        "AllGather", OP.bypass,
                        replica_groups=[list(range(n_cores))],
                        ins=[HsLocal[:, :].opt()],
                        outs=[HsFull[:, :].opt()])
                else:
                    for cc_i in range(n_cores):
                        nc.sync.dma_start(
                            out=HsFull[cc_i * N_loc:(cc_i + 1) * N_loc, :],
                            in_=HsLocal[:, :])
                if dbg_stage == f"hsf{l}":
                    nc.sync.dma_start(out=dbg2_d[:, :], in_=HsFull[:, :])
                if l == 3:
                    H3 = hpool.tile([P, T * DO], F32, name="H3", tag="hx")
                phase2(l, HsFull, H3)
                if dbg_stage == f"h{l}":
                    dump_dbg(H if l < 3 else H3, None if l < 3 else T * DO)

            # ---- global add pool
            if max_layers < 3:
                zz = stage.tile([P, DO], F32, name="zz", tag="ost")
                nc.gpsimd.memset(zz[:], 0.0)
                for w in range(GW):
                    nc.sync.dma_start(out=out_d[w * P:(w + 1) * P, :],
                                      in_=zz[:])
                return nc
            for w in range(GW):
                pp = pt.tile([P, DO], F32, name="pp", tag="pp")
                for t in range(T):
                    sp = spool.tile([P, P], F32, name="sp", tag="sp")
                    nc.vector.tensor_tensor(
                        out=sp[:],
                        in0=poolid_sb[:, w * T + t:w * T + t + 1]
                            .to_broadcast([P, P]),
                        in1=iota_sb[:, :], op=OP.is_equal)
                    nc.tensor.matmul(pp[:], lhsT=sp[:],
                                     rhs=H3[:, t * DO:(t + 1) * DO],
                                     start=(t == 0), stop=(t == T - 1))
                ost = stage.tile([P, DO], F32, name="ost", tag="ost")
                nc.vector.tensor_copy(out=ost[:], in_=pp[:])
                nc.sync.dma_start(out=out_d[w * P:(w + 1) * P, :],
                                  in_=ost[:])

    return nc


# ----------------------------------------------------------------------------
# Driver
# ----------------------------------------------------------------------------

def _run(x, edge_index, batch, W_emb, b_emb, W1, b1, W2, b2, W3, b3,
         G=G_TOTAL, n_cores=N_CORES, trace=False):
    x = np.ascontiguousarray(np.asarray(x, dtype=np.float32))
    edge_index = np.ascontiguousarray(np.asarray(edge_index, dtype=np.int64))
    batch_np = np.ascontiguousarray(np.asarray(batch, dtype=np.int64))

    cfg, in_maps, bounds, g_lo, g_cnt = _preprocess(
        x, edge_index, batch_np, n_cores, G)

    def bpad(b):
        v = np.zeros((P, 1), dtype=np.float32)
        b = np.asarray(b, dtype=np.float32).reshape(-1)
        v[:b.shape[0], 0] = b
        return v

    shared = dict(
        W0=np.asarray(W_emb, dtype=np.float32),
        W1=np.asarray(W1, dtype=np.float32),
        W2=np.asarray(W2, dtype=np.float32),
        W3=np.asarray(W3, dtype=np.float32),
        b0=bpad(b_emb), b1=bpad(b1), b2=bpad(b2), b3=bpad(b3))
    for m in in_maps:
        m.update(shared)

    nc = _build_program(cfg)
    nc.finalize()
    res = run_bass_kernel_spmd(nc, in_maps, list(range(n_cores)),
                               trace=trace)

    out = np.zeros((G, 64), dtype=np.float32)
    for c in range(n_cores):
        oc = np.asarray(res.results[c]["out"])
        if g_cnt[c] > 0:
            out[g_lo[c]:g_lo[c] + g_cnt[c]] = oc[:g_cnt[c]]
    return out, res


def kernel(**inputs):
    out, _ = _run(G=G_TOTAL, n_cores=N_CORES,
                  trace=bool(int(os.environ.get("GCN_TRACE", "0"))),
                  **inputs)
    return out



# revision 3
# speedup vs baseline: 1.4002x; 1.4002x over previous
"""Trainium2 Bass kernel for a 3-layer GCN (Kipf-Welling, symmetric norm,
self-loops) with global add pooling.

Distribution: nodes (graph-aligned contiguous ranges) are sharded across 8
NeuronCores.  Each core owns the aggregation (scatter-add) for its local dst
nodes; the per-layer activations are exchanged with an AllGather so every core
can gather arbitrary source rows with indirect DMA.

Math (matches the jax reference exactly):
    deg  = indeg + 1, dis = deg^-1/2
    Hs   = dis * (H @ W)              (rows scaled by dis)
    agg  = dis_dst * sum_{e:(s->d)} Hs[s]   over edges *including self-loops*
         = sum_e dis_s dis_d (HW)[s] + (HW)[d]/deg_d
    H'   = relu(agg + b)              (no relu on layer 3)
    out  = segment_sum(H3, batch)

Feature-major layout on chip: H^T tiles [128 feats, nodes] so the layer
matmul streams with W as the stationary operand.  The edge scatter-add is a
matmul with an on-the-fly selection matrix S[e, d] = (dst_id[e] == d), built
on the vector engine by comparing per-edge dst ids against an iota row.
"""

import os
import sys
import math

import numpy as np

sys.path.insert(0, "/opt/trn_rl_repo")

import concourse.bass as bass  # noqa: E402
import concourse.bacc as bacc  # noqa: E402
import concourse.tile as tile  # noqa: E402
from concourse import mybir  # noqa: E402
from concourse.bass_utils import run_bass_kernel_spmd  # noqa: E402
from concourse.masks import make_identity  # noqa: E402

P = 128
F32 = mybir.dt.float32
F16 = mybir.dt.float16
I32 = mybir.dt.int32
I16 = mybir.dt.int16
OP = mybir.AluOpType

N_CORES = 8
G_TOTAL = 1000  # graphs in the batch (fixed by the problem)


# ----------------------------------------------------------------------------
# Host-side preprocessing: shard nodes/edges, build gather/selection metadata.
# ----------------------------------------------------------------------------

def _preprocess(x, edge_index, batch, n_cores, G):
    N = x.shape[0]
    src = edge_index[0].astype(np.int64)
    dst = edge_index[1].astype(np.int64)
    batch = batch.astype(np.int64)

    # graph-aligned shard boundaries near equal node counts
    graph_start = np.searchsorted(batch, np.arange(G + 1))  # [G+1], node idx
    bounds = [0]
    for c in range(1, n_cores):
        target = (c * N) // n_cores
        gi = np.searchsorted(graph_start, target)
        lo = graph_start[gi - 1] if gi > 0 else 0
        hi = graph_start[gi] if gi <= G else N
        b = int(hi if (hi - target) <= (target - lo) else lo)
        b = max(b, bounds[-1])  # keep non-decreasing
        bounds.append(b)
    bounds.append(N)
    bounds = np.asarray(bounds, dtype=np.int64)

    shard_sizes = bounds[1:] - bounds[:-1]
    N_loc = int(math.ceil(int(shard_sizes.max()) / P) * P)
    T = N_loc // P

    # normalization (index-derived scalars)
    deg = np.bincount(dst, minlength=N).astype(np.float32) + np.float32(1.0)
    dis = (np.float32(1.0) / np.sqrt(deg)).astype(np.float32)

    # padded-global source row ids (rows of the allgathered Hs table)
    core_of = np.searchsorted(bounds, dst, side="right") - 1
    core_of_src = np.searchsorted(bounds, src, side="right") - 1
    src_pg = core_of_src * N_loc + (src - bounds[core_of_src])

    # src-table quarters: int16 gather indices must stay < 32768
    QC = 2                      # cores per quarter
    NQ = n_cores // QC          # quarters
    QR = QC * N_loc             # rows per quarter
    TB = 2                      # tiles per gather block
    assert QR <= 32767, (QR, N_loc)

    per_core = []
    CPS = 1
    for c in range(n_cores):
        n_real = int(bounds[c + 1] - bounds[c])
        m = core_of == c
        dstl = np.concatenate([dst[m] - bounds[c], np.arange(n_real)])
        srcs = np.concatenate([src_pg[m], c * N_loc + np.arange(n_real)])
        tile_id = dstl // P
        quarter = srcs // QR
        key = tile_id * NQ + quarter
        order = np.argsort(key, kind="stable")
        dstl, srcs, tile_id, quarter, key = (
            dstl[order], srcs[order], tile_id[order], quarter[order],
            key[order])
        counts = np.bincount(key, minlength=T * NQ)
        CPS = max(CPS, int(math.ceil(int(counts.max()) / P)))
        per_core.append((n_real, dstl, srcs, key, counts))

    NCH = T * NQ * CPS          # total chunks per core
    in_maps = []
    g_lo = []
    g_cnt = []
    GW = None
    for c in range(n_cores):
        n_real, dstl, srcs, key, counts = per_core[c]
        # slot grid: edge k of (tile,quarter) group -> chunk k//P, part k%P
        goff = np.concatenate([[0], np.cumsum(counts)])[:-1]
        rank = np.arange(dstl.shape[0]) - goff[key]
        chunk = rank // P
        part = rank % P
        col = key * CPS + chunk            # global chunk column (t, q, c)
        tile_id = key // NQ

        # int16 gather indices in gather-group order:
        # group (tile-block tb, quarter q) -> flat j = (t_loc*CPS+c)*128+p.
        # 16-partition-wrapped within each group, replicated x8 core groups.
        quarter = key % NQ
        tb = tile_id // TB
        t_loc = tile_id % TB
        tbg = np.minimum(TB, T - tb * TB)  # tiles in this block
        block_base = tb * NQ * TB * CPS    # chunk cols before this block
        gcol = block_base + quarter * tbg * CPS + t_loc * CPS + chunk
        flat = gcol * P + part
        idx16 = np.zeros((16, NCH * P // 16), dtype=np.int16)
        idx16[flat % 16, flat // 16] = (srcs % QR).astype(np.int16)
        srcidx = np.tile(idx16, (8, 1))
        dstid = np.full((P, NCH), 1.0e6, dtype=np.float32)
        dstid[part, col] = (dstl - tile_id * P).astype(np.float32)

        dis_loc = np.ones(N_loc, dtype=np.float32)
        dis_loc[:n_real] = dis[bounds[c]:bounds[c + 1]]
        disrep = np.broadcast_to(dis_loc.astype(np.float16), (P, N_loc)).copy()

        xT = np.zeros((P, N_loc), dtype=np.float32)
        xT[:, :n_real] = x[bounds[c]:bounds[c + 1]].T

        bloc = batch[bounds[c]:bounds[c + 1]]
        glo = int(bloc[0]) if n_real > 0 else 0
        gct = int(bloc[-1]) + 1 - glo if n_real > 0 else 0
        g_lo.append(glo)
        g_cnt.append(gct)
        in_maps.append(dict(srcidx=srcidx, dstid=dstid, disrep=disrep, xT=xT,
                            _bloc=bloc - glo, _n_real=n_real))

    GW = max(1, int(math.ceil(max(g_cnt) / P)))
    iota = np.broadcast_to(np.arange(P, dtype=np.float32), (P, P)).copy()
    for c in range(n_cores):
        d = in_maps[c]
        bloc, n_real = d.pop("_bloc"), d.pop("_n_real")
        poolid = np.full((P, T * GW), 1.0e6, dtype=np.float32)
        j = np.arange(n_real)
        for w in range(GW):
            poolid[j % P, (j // P) + w * T] = (bloc - w * P).astype(np.float32)
        d["poolid"] = poolid
        d["iota"] = iota

    cfg = dict(T=T, CPS=CPS, NQ=NQ, QR=QR, TB=TB, GW=GW, N_loc=N_loc,
               n_cores=n_cores)
    return cfg, in_maps, bounds, g_lo, g_cnt


# ----------------------------------------------------------------------------
# Bass program
# ----------------------------------------------------------------------------

def _build_program(cfg):
    T, CPS, GW, N_loc = cfg["T"], cfg["CPS"], cfg["GW"], cfg["N_loc"]
    NQ, QR, TB = cfg["NQ"], cfg["QR"], cfg["TB"]
    n_cores = cfg["n_cores"]
    NCH = T * NQ * CPS
    D, DO = 128, 64
    DOUT = {1: D, 2: D, 3: DO}

    nc = bacc.Bacc(None, num_devices=n_cores, num_swdge_queues=4)

    xT_d = nc.dram_tensor("xT", [P, N_loc], F32, kind="ExternalInput")
    W_d = {0: nc.dram_tensor("W0", [D, D], F32, kind="ExternalInput"),
           1: nc.dram_tensor("W1", [D, D], F32, kind="ExternalInput"),
           2: nc.dram_tensor("W2", [D, D], F32, kind="ExternalInput"),
           3: nc.dram_tensor("W3", [D, DO], F32, kind="ExternalInput")}
    b_d = {l: nc.dram_tensor(f"b{l}", [P, 1], F32, kind="ExternalInput")
           for l in range(4)}
    srcidx_d = nc.dram_tensor("srcidx", [P, NCH * P // 16], I16,
                              kind="ExternalInput")
    dstid_d = nc.dram_tensor("dstid", [P, NCH], F32, kind="ExternalInput")
    disrep_d = nc.dram_tensor("disrep", [P, N_loc], F16, kind="ExternalInput")
    poolid_d = nc.dram_tensor("poolid", [P, T * GW], F32, kind="ExternalInput")
    iota_d = nc.dram_tensor("iota", [P, P], F32, kind="ExternalInput")
    out_d = nc.dram_tensor("out", [GW * P, DO], F32, kind="ExternalOutput")
    dbg_stage = os.environ.get("GCN_DBG_STAGE", "")
    dbg_d = dbg2_d = None
    if dbg_stage.startswith("h"):
        dbg_d = nc.dram_tensor("dbg", [P, N_loc], F32, kind="ExternalOutput")
    if dbg_stage.startswith("hsf"):
        dbg2_d = nc.dram_tensor("dbg2", [n_cores * N_loc, D], F32,
                                kind="ExternalOutput")

    with tile.TileContext(nc) as tc:
        with tc.tile_pool(name="const", bufs=1) as const, \
             tc.tile_pool(name="hpool", bufs=1) as hpool, \
             tc.tile_pool(name="stage", bufs=3) as stage, \
             tc.tile_pool(name="rpool", bufs=NQ + 1) as rpool, \
             tc.tile_pool(name="spool", bufs=1) as spool, \
             tc.tile_pool(name="dram", bufs=2, space="DRAM") as dram, \
             tc.tile_pool(name="pm", bufs=2, space="PSUM") as pm, \
             tc.tile_pool(name="pt", bufs=2, space="PSUM") as pt, \
             tc.tile_pool(name="pa", bufs=2, space="PSUM") as pa:

            # ---- constants into SBUF
            w_sb = {}
            for l in range(4):
                w = const.tile([D, DOUT.get(l, D) if l else D], F32,
                               name=f"w{l}sb")
                nc.sync.dma_start(out=w[:], in_=W_d[l][:, :])
                w_sb[l] = w
            b_sb = {}
            for l in range(4):
                b = const.tile([P, 1], F32, name=f"b{l}sb")
                nc.sync.dma_start(out=b[:], in_=b_d[l][:, :])
                b_sb[l] = b
            gmin = int(os.environ.get("GCN_MIN", "0"))
            iota_sb = const.tile([P, P], F32, name="iotasb")
            nc.sync.dma_start(out=iota_sb[:], in_=iota_d[:, :])
            iden_sb = const.tile([P, P], F32, name="idensb")
            if not gmin:
                make_identity(nc, iden_sb[:])
            srcidx_sb = const.tile([P, NCH * P // 16], I16, name="srcidxsb")
            nc.sync.dma_start(out=srcidx_sb[:], in_=srcidx_d[:, :])
            dstid_sb = const.tile([P, NCH], F32, name="dstidsb")
            disrep_sb = const.tile([P, N_loc], F16, name="disrepsb")
            poolid_sb = const.tile([P, T * GW], F32, name="poolidsb")
            if not gmin:
                nc.sync.dma_start(out=dstid_sb[:], in_=dstid_d[:, :])
                nc.sync.dma_start(out=disrep_sb[:], in_=disrep_d[:, :])
                nc.sync.dma_start(out=poolid_sb[:], in_=poolid_d[:, :])

            H = hpool.tile([P, N_loc], F32, name="H", tag="ha")

            # Join all const-load DMA sems into the DVE engine clock so later
            # DVE tensor_tensor ops (2 sync-wait slots in the ISA) don't have
            # to carry per-DMA waits themselves.
            if not gmin:
                joiner = const.tile([P, 1], F32, name="joiner")
                for cst in [iota_sb, dstid_sb, disrep_sb, poolid_sb,
                            b_sb[0], b_sb[1], b_sb[2], b_sb[3]]:
                    nc.vector.tensor_copy(out=joiner[:, :1], in_=cst[:, :1])

            # ---- phase 1: M^T = W^T @ Hprev^T, scale by dis, transpose,
            #      write node-major Hs to local DRAM (skipped for l=0/emb).
            def phase1(l, HprevT, HsLocal):
                dout = DOUT.get(l, D)
                nk = (N_loc + 511) // 512
                for k in range(nk):
                    c0 = k * 512
                    cw = min(512, N_loc - c0)
                    if l == 0:
                        # stream x^T chunks from DRAM (saves 6.5MB SBUF)
                        xst = stage.tile([P, 512], F32, name="xst", tag="ms")
                        nc.sync.dma_start(out=xst[:, :cw],
                                          in_=xT_d[:, c0:c0 + cw])
                        rhs_ap = xst[:, :cw]
                    else:
                        rhs_ap = HprevT[:, c0:c0 + cw]
                    mm = pm.tile([P, 512], F32, name="mm", tag="pm")
                    nc.tensor.matmul(mm[:dout, :cw], lhsT=w_sb[l][:, :dout],
                                     rhs=rhs_ap,
                                     start=True, stop=True)
                    if l == 0:
                        nc.vector.tensor_scalar(
                            out=H[:, c0:c0 + cw], in0=mm[:, :cw],
                            scalar1=b_sb[0][:, :], scalar2=None, op0=OP.add)
                        continue
                    ms = stage.tile([P, 512], F32, name="ms", tag="ms")
                    nc.vector.tensor_tensor(
                        out=ms[:dout, :cw], in0=mm[:dout, :cw],
                        in1=disrep_sb[:dout, c0:c0 + cw], op=OP.mult)
                    for tt in range(cw // P):
                        tglob = k * 4 + tt
                        ptt = pt.tile([P, P], F32, name="ptt", tag="pt")
                        nc.tensor.transpose(
                            out=ptt[:, :dout],
                            in_=ms[:dout, tt * P:(tt + 1) * P],
                            identity=iden_sb[:dout, :dout])
                        hs = stage.tile([P, P], F32, name="hs", tag="hs")
                        nc.vector.tensor_copy(out=hs[:, :dout],
                                              in_=ptt[:, :dout])
                        nc.sync.dma_start(
                            out=HsLocal[tglob * P:(tglob + 1) * P, :],
                            in_=hs[:, :dout])

            # ---- phase 2: gather + scatter-add into local dst tiles.
            def phase2(l, HsFull, H3):
                dout = DOUT[l]
                NB = (T + TB - 1) // TB
                for tb in range(NB):
                    tbg = min(TB, T - tb * TB)
                    block_base = tb * NQ * TB * CPS
                    num = tbg * CPS * P
                    Rq = []
                    for q in range(NQ):
                        R = rpool.tile([P, TB * CPS * dout], F32,
                                       name="R", tag="R")
                        c0 = (block_base + q * tbg * CPS) * P // 16
                        gi = phase2.gcount = getattr(phase2, "gcount", 0) + 1
                        real_n = int(os.environ.get("GCN_REAL_GATHERS",
                                                    "99999"))
                        if (int(os.environ.get("GCN_FAKE_GATHER", "0"))
                                or gi > real_n):
                            nc.sync.dma_start(
                                out=R[:, :tbg * CPS * dout],
                                in_=HsFull[q * QR:
                                           q * QR + P * tbg * CPS,
                                           :].rearrange(
                                    "(p c) e -> p (c e)", p=P))
                        else:
                            nc.gpsimd.dma_gather(
                                out_ap=R[:, :tbg * CPS * dout].rearrange(
                                    "p (c e) -> p c e", e=dout),
                                in_ap=HsFull[q * QR:(q + 1) * QR, :],
                                idxs_ap=srcidx_sb[:, c0:c0 + num // 16],
                                num_idxs=num,
                                num_idxs_reg=num,
                                elem_size=dout,
                                single_packet=(num <= 1008),
                                queue_num=q % 4)
                        Rq.append(R)
                    if int(os.environ.get("GCN_NO_AGG", "0")):
                        for q in range(NQ):
                            nc.vector.tensor_copy(
                                out=H[:, (tb * NQ + q) % N_loc:
                                      (tb * NQ + q) % N_loc + 1],
                                in_=Rq[q][:, 0:1])
                        continue
                    for tl in range(tbg):
                        phase2_tile(l, tb * TB + tl, tl, Rq, H3)

            def phase2_tile(l, t, tl, Rq, H3):
                dout = DOUT[l]
                nch = NQ * CPS
                S = spool.tile([P, NQ * CPS * P], F32, name="S", tag="S")
                nc.vector.tensor_tensor(
                    out=S[:].rearrange("p (c d) -> p c d", d=P),
                    in0=dstid_sb[:, t * nch:(t + 1) * nch]
                        .unsqueeze(2).broadcast_to([P, nch, P]),
                    in1=iota_sb[:, :].unsqueeze(1).broadcast_to([P, nch, P]),
                    op=OP.is_equal)
                agg = pa.tile([P, P], F32, name="agg", tag="pa")
                for q in range(NQ):
                    for c in range(CPS):
                        k = q * CPS + c
                        nc.tensor.matmul(
                            agg[:dout, :],
                            lhsT=Rq[q][:, (tl * CPS + c) * dout:
                                       (tl * CPS + c + 1) * dout],
                            rhs=S[:, k * P:(k + 1) * P],
                            start=(k == 0), stop=(k == nch - 1))
                tmp = stage.tile([P, P], F32, name="tmp", tag="tmp")
                nc.vector.tensor_tensor(
                    out=tmp[:dout, :], in0=agg[:dout, :],
                    in1=disrep_sb[:dout, t * P:(t + 1) * P], op=OP.mult)
                if l < 3:
                    nc.vector.tensor_scalar(
                        out=H[:, t * P:(t + 1) * P], in0=tmp[:, :],
                        scalar1=b_sb[l][:, :], scalar2=0.0,
                        op0=OP.add, op1=OP.max)
                else:
                    t2 = stage.tile([P, P], F32, name="t2", tag="tmp")
                    nc.vector.tensor_scalar(
                        out=t2[:dout, :], in0=tmp[:dout, :],
                        scalar1=b_sb[3][:dout, :], scalar2=None,
                        op0=OP.add)
                    ptt = pt.tile([P, P], F32, name="ptt2", tag="pt")
                    nc.tensor.transpose(
                        out=ptt[:, :dout], in_=t2[:dout, :],
                        identity=iden_sb[:dout, :dout])
                    nc.vector.tensor_copy(
                        out=H3[:, t * DO:(t + 1) * DO],
                        in_=ptt[:, :dout])

            def dump_dbg(buf, width=None):
                nc.sync.dma_start(out=dbg_d[:, :width] if width else dbg_d[:, :],
                                  in_=buf[:, :width] if width else buf[:, :])

            # ---- the network
            max_layers = int(os.environ.get("GCN_MAX_LAYERS", "3"))
            skip_p1 = int(os.environ.get("GCN_SKIP_P1", "0"))
            if not skip_p1:
                phase1(0, None, None)  # embedding -> H (streams xT from DRAM)
            if dbg_stage == "h0":
                dump_dbg(H)
            H3 = None
            for l in range(1, max_layers + 1):
                dout = DOUT[l]
                HsLocal = dram.tile([N_loc, dout], F32, name=f"hsl{l}",
                                    tag="hsl")
                if skip_p1:
                    for r in range(T):
                        hz = stage.tile([P, P], F32, name="hz", tag="hs")
                        nc.vector.memset(hz[:, :dout], 1.0)
                        nc.sync.dma_start(
                            out=HsLocal[r * P:(r + 1) * P, :],
                            in_=hz[:, :dout])
                else:
                    phase1(l, H, HsLocal)
                HsFull = dram.tile(
                    [n_cores * N_loc, dout], F32, name=f"hsf{l}", tag="hsf",
                    addr_space="Local"
                    if int(os.environ.get("GCN_LOCAL_HSF", "0"))
                    else "Shared")
                if n_cores > 1 and not int(os.environ.get("GCN_NO_CC", "0")):
                    nc.gpsimd.collective_compute(
                        "AllGather", OP.bypass,
                        replica_groups=[list(range(n_cores))],
                        ins=[HsLocal[:, :].opt()],
                        outs=[HsFull[:, :].opt()])
                else:
                    for cc_i in range(n_cores):
                        nc.sync.dma_start(
                            out=HsFull[cc_i * N_loc:(cc_i + 1) * N_loc, :],
                            in_=HsLocal[:, :])
                if dbg_stage == f"hsf{l}":
                    nc.sync.dma_start(out=dbg2_d[:, :], in_=HsFull[:, :])
                if l == 3:
                    H3 = hpool.tile([P, T * DO], F32, name="H3", tag="hx")
                phase2(l, HsFull, H3)
                if dbg_stage == f"h{l}":
                    dump_dbg(H if l < 3 else H3, None if l < 3 else T * DO)

            # ---- global add pool
            if max_layers < 3:
                zz = stage.tile([P, DO], F32, name="zz", tag="ost")
                nc.gpsimd.memset(zz[:], 0.0)
                for w in range(GW):
                    nc.sync.dma_start(out=out_d[w * P:(w + 1) * P, :],
                                      in_=zz[:])
                return nc
            for w in range(GW):
                pp = pt.tile([P, DO], F32, name="pp", tag="pp")
                for t in range(T):
                    sp = spool.tile([P, P], F32, name="sp", tag="sp")
                    nc.vector.tensor_tensor(
                        out=sp[:],
                        in0=poolid_sb[:, w * T + t:w * T + t + 1]
                            .to_broadcast([P, P]),
                        in1=iota_sb[:, :], op=OP.is_equal)
                    nc.tensor.matmul(pp[:], lhsT=sp[:],
                                     rhs=H3[:, t * DO:(t + 1) * DO],
                                     start=(t == 0), stop=(t == T - 1))
                ost = stage.tile([P, DO], F32, name="ost", tag="ost")
                nc.vector.tensor_copy(out=ost[:], in_=pp[:])
                nc.sync.dma_start(out=out_d[w * P:(w + 1) * P, :],
                                  in_=ost[:])

    return nc


# ----------------------------------------------------------------------------
# Driver
# ----------------------------------------------------------------------------

def _run(x, edge_index, batch, W_emb, b_emb, W1, b1, W2, b2, W3, b3,
         G=G_TOTAL, n_cores=N_CORES, trace=False):
    x = np.ascontiguousarray(np.asarray(x, dtype=np.float32))
    edge_index = np.ascontiguousarray(np.asarray(edge_index, dtype=np.int64))
    batch_np = np.ascontiguousarray(np.asarray(batch, dtype=np.int64))

    cfg, in_maps, bounds, g_lo, g_cnt = _preprocess(
        x, edge_index, batch_np, n_cores, G)

    def bpad(b):
        v = np.zeros((P, 1), dtype=np.float32)
        b = np.asarray(b, dtype=np.float32).reshape(-1)
        v[:b.shape[0], 0] = b
        return v

    shared = dict(
        W0=np.asarray(W_emb, dtype=np.float32),
        W1=np.asarray(W1, dtype=np.float32),
        W2=np.asarray(W2, dtype=np.float32),
        W3=np.asarray(W3, dtype=np.float32),
        b0=bpad(b_emb), b1=bpad(b1), b2=bpad(b2), b3=bpad(b3))
    for m in in_maps:
        m.update(shared)

    nc = _build_program(cfg)
    nc.finalize()
    res = run_bass_kernel_spmd(nc, in_maps, list(range(n_cores)),
                               trace=trace)

    out = np.zeros((G, 64), dtype=np.float32)
    for c in range(n_cores):
        oc = np.asarray(res.results[c]["out"])
        if g_cnt[c] > 0:
            out[g_lo[c]:g_lo[c] + g_cnt[c]] = oc[:g_cnt[c]]
    return out, res


def kernel(**inputs):
    out, _ = _run(G=G_TOTAL, n_cores=N_CORES,
                  trace=bool(int(os.environ.get("GCN_TRACE", "0"))),
                  **inputs)
    return out



# revision 16
# speedup vs baseline: 2.4811x; 1.7720x over previous
"""Trainium2 Bass kernel for a 3-layer GCN (Kipf-Welling, symmetric norm,
self-loops) with global add pooling.

Distribution: nodes (graph-aligned contiguous ranges) are sharded across 8
NeuronCores.  Each core owns the aggregation (scatter-add) for its local dst
nodes; the per-layer activations are exchanged with an AllGather so every core
can gather arbitrary source rows with indirect DMA.

Math (matches the jax reference exactly):
    deg  = indeg + 1, dis = deg^-1/2
    Hs   = dis * (H @ W)              (rows scaled by dis)
    agg  = dis_dst * sum_{e:(s->d)} Hs[s]   over edges *including self-loops*
         = sum_e dis_s dis_d (HW)[s] + (HW)[d]/deg_d
    H'   = relu(agg + b)              (no relu on layer 3)
    out  = segment_sum(H3, batch)

Feature-major layout on chip: H^T tiles [128 feats, nodes] so the layer
matmul streams with W as the stationary operand.  The edge scatter-add is a
matmul with an on-the-fly selection matrix S[e, d] = (dst_id[e] == d), built
on the vector engine by comparing per-edge dst ids against an iota row.
"""

import os
import sys
import math

import numpy as np

sys.path.insert(0, "/opt/trn_rl_repo")

import concourse.bass as bass  # noqa: E402
import concourse.bacc as bacc  # noqa: E402
import concourse.tile as tile  # noqa: E402
from concourse import mybir  # noqa: E402
from concourse.bass_utils import run_bass_kernel_spmd  # noqa: E402
from concourse.masks import make_identity  # noqa: E402

P = 128
F32 = mybir.dt.float32
F16 = mybir.dt.float16
I32 = mybir.dt.int32
I16 = mybir.dt.int16
OP = mybir.AluOpType

N_CORES = 8
G_TOTAL = 1000  # graphs in the batch (fixed by the problem)


# ----------------------------------------------------------------------------
# Host-side preprocessing: shard nodes/edges, build gather/selection metadata.
# ----------------------------------------------------------------------------

def _preprocess(x, edge_index, batch, n_cores, G):
    N = x.shape[0]
    src = edge_index[0].astype(np.int64)
    dst = edge_index[1].astype(np.int64)
    batch = batch.astype(np.int64)

    # graph-aligned shard boundaries near equal node counts
    graph_start = np.searchsorted(batch, np.arange(G + 1))  # [G+1], node idx
    bounds = [0]
    for c in range(1, n_cores):
        target = (c * N) // n_cores
        gi = np.searchsorted(graph_start, target)
        lo = graph_start[gi - 1] if gi > 0 else 0
        hi = graph_start[gi] if gi <= G else N
        b = int(hi if (hi - target) <= (target - lo) else lo)
        b = max(b, bounds[-1])  # keep non-decreasing
        bounds.append(b)
    bounds.append(N)
    bounds = np.asarray(bounds, dtype=np.int64)

    shard_sizes = bounds[1:] - bounds[:-1]
    N_loc = int(math.ceil(int(shard_sizes.max()) / P) * P)
    T = N_loc // P

    # normalization (index-derived scalars)
    deg = np.bincount(dst, minlength=N).astype(np.float32) + np.float32(1.0)
    dis = (np.float32(1.0) / np.sqrt(deg)).astype(np.float32)

    # padded-global source row ids (rows of the allgathered Hs table)
    core_of = np.searchsorted(bounds, dst, side="right") - 1
    core_of_src = np.searchsorted(bounds, src, side="right") - 1
    src_pg = core_of_src * N_loc + (src - bounds[core_of_src])

    # src-table quarters: int16 gather indices must stay < 32768
    QC = 2                      # cores per quarter
    NQ = n_cores // QC          # quarters
    QR = QC * N_loc             # rows per quarter
    TB = 2                      # tiles per gather block
    assert QR <= 32767, (QR, N_loc)

    per_core = []
    CPS = 1
    for c in range(n_cores):
        n_real = int(bounds[c + 1] - bounds[c])
        m = core_of == c
        dstl = np.concatenate([dst[m] - bounds[c], np.arange(n_real)])
        srcs = np.concatenate([src_pg[m], c * N_loc + np.arange(n_real)])
        tile_id = dstl // P
        quarter = srcs // QR
        key = tile_id * NQ + quarter
        order = np.argsort(key, kind="stable")
        dstl, srcs, tile_id, quarter, key = (
            dstl[order], srcs[order], tile_id[order], quarter[order],
            key[order])
        counts = np.bincount(key, minlength=T * NQ)
        CPS = max(CPS, int(math.ceil(int(counts.max()) / P)))
        per_core.append((n_real, dstl, srcs, key, counts))

    NCH = T * NQ * CPS          # total chunks per core
    in_maps = []
    g_lo = []
    g_cnt = []
    GW = None
    for c in range(n_cores):
        n_real, dstl, srcs, key, counts = per_core[c]
        # slot grid: edge k of (tile,quarter) group -> chunk k//P, part k%P
        goff = np.concatenate([[0], np.cumsum(counts)])[:-1]
        rank = np.arange(dstl.shape[0]) - goff[key]
        chunk = rank // P
        part = rank % P
        col = key * CPS + chunk            # global chunk column (t, q, c)
        tile_id = key // NQ

        # int16 gather indices in gather-group order:
        # group (tile-block tb, quarter q) -> flat j = (t_loc*CPS+c)*128+p.
        # 16-partition-wrapped within each group, replicated x8 core groups.
        quarter = key % NQ
        tb = tile_id // TB
        t_loc = tile_id % TB
        tbg = np.minimum(TB, T - tb * TB)  # tiles in this block
        block_base = tb * NQ * TB * CPS    # chunk cols before this block
        gcol = block_base + quarter * tbg * CPS + t_loc * CPS + chunk
        flat = gcol * P + part
        idx16 = np.zeros((16, NCH * P // 16), dtype=np.int16)
        idx16[flat % 16, flat // 16] = (srcs % QR).astype(np.int16)
        srcidx = np.tile(idx16, (8, 1))
        dstid = np.full((P, NCH), 1.0e6, dtype=np.float32)
        dstid[part, col] = (dstl - tile_id * P).astype(np.float32)

        dis_loc = np.ones(N_loc, dtype=np.float32)
        dis_loc[:n_real] = dis[bounds[c]:bounds[c + 1]]
        disrep = np.broadcast_to(dis_loc.astype(np.float16), (P, N_loc)).copy()

        xT = np.zeros((P, N_loc), dtype=np.float32)
        xT[:, :n_real] = x[bounds[c]:bounds[c + 1]].T

        bloc = batch[bounds[c]:bounds[c + 1]]
        glo = int(bloc[0]) if n_real > 0 else 0
        gct = int(bloc[-1]) + 1 - glo if n_real > 0 else 0
        g_lo.append(glo)
        g_cnt.append(gct)
        in_maps.append(dict(srcidx=srcidx, dstid=dstid, disrep=disrep, xT=xT,
                            _bloc=bloc - glo, _n_real=n_real))

    GW = max(1, int(math.ceil(max(g_cnt) / P)))
    iota = np.broadcast_to(np.arange(P, dtype=np.float32), (P, P)).copy()
    for c in range(n_cores):
        d = in_maps[c]
        bloc, n_real = d.pop("_bloc"), d.pop("_n_real")
        poolid = np.full((P, T * GW), 1.0e6, dtype=np.float32)
        j = np.arange(n_real)
        for w in range(GW):
            poolid[j % P, (j // P) + w * T] = (bloc - w * P).astype(np.float32)
        d["poolid"] = poolid
        d["iota"] = iota

    cfg = dict(T=T, CPS=CPS, NQ=NQ, QR=QR, TB=TB, GW=GW, N_loc=N_loc,
               n_cores=n_cores)
    return cfg, in_maps, bounds, g_lo, g_cnt


# ----------------------------------------------------------------------------
# Bass program
# ----------------------------------------------------------------------------

def _build_program(cfg):
    T, CPS, GW, N_loc = cfg["T"], cfg["CPS"], cfg["GW"], cfg["N_loc"]
    NQ, QR, TB = cfg["NQ"], cfg["QR"], cfg["TB"]
    n_cores = cfg["n_cores"]
    NCH = T * NQ * CPS
    D, DO = 128, 64
    # layer 3 is zero-padded to 128 output features so fp16 table rows stay
    # 256B (dma_gather elem_size restriction) and the message path is uniform
    DOUT = {1: D, 2: D, 3: D}

    nc = bacc.Bacc(None, num_devices=n_cores, num_swdge_queues=4)

    xT_d = nc.dram_tensor("xT", [P, N_loc], F32, kind="ExternalInput")
    W_d = {0: nc.dram_tensor("W0", [D, D], F32, kind="ExternalInput"),
           1: nc.dram_tensor("W1", [D, D], F32, kind="ExternalInput"),
           2: nc.dram_tensor("W2", [D, D], F32, kind="ExternalInput"),
           3: nc.dram_tensor("W3", [D, D], F32, kind="ExternalInput")}
    b_d = {l: nc.dram_tensor(f"b{l}", [P, 1], F32, kind="ExternalInput")
           for l in range(4)}
    srcidx_d = nc.dram_tensor("srcidx", [P, NCH * P // 16], I16,
                              kind="ExternalInput")
    dstid_d = nc.dram_tensor("dstid", [P, NCH], F32, kind="ExternalInput")
    disrep_d = nc.dram_tensor("disrep", [P, N_loc], F16, kind="ExternalInput")
    poolid_d = nc.dram_tensor("poolid", [P, T * GW], F32, kind="ExternalInput")
    iota_d = nc.dram_tensor("iota", [P, P], F32, kind="ExternalInput")
    out_d = nc.dram_tensor("out", [GW * P, DO], F32, kind="ExternalOutput")
    dbg_stage = os.environ.get("GCN_DBG_STAGE", "")
    dbg_d = dbg2_d = None
    if dbg_stage.startswith("h"):
        dbg_d = nc.dram_tensor("dbg", [P, N_loc], F32, kind="ExternalOutput")
    if dbg_stage.startswith("hsf"):
        dbg2_d = nc.dram_tensor("dbg2", [n_cores * N_loc, D], F16,
                                kind="ExternalOutput")

    with tile.TileContext(nc) as tc:
        with tc.tile_pool(name="const", bufs=1) as const, \
             tc.tile_pool(name="hpool", bufs=1) as hpool, \
             tc.tile_pool(name="stage", bufs=3) as stage, \
             tc.tile_pool(name="rpool", bufs=2 * NQ) as rpool, \
             tc.tile_pool(name="spool", bufs=1) as spool, \
             tc.tile_pool(name="dram", bufs=2, space="DRAM") as dram, \
             tc.tile_pool(name="pm", bufs=2, space="PSUM") as pm, \
             tc.tile_pool(name="pt", bufs=2, space="PSUM") as pt, \
             tc.tile_pool(name="pa", bufs=2, space="PSUM") as pa:

            # ---- constants into SBUF
            w_sb = {}
            for l in range(4):
                w = const.tile([D, DOUT.get(l, D) if l else D], F32,
                               name=f"w{l}sb")
                nc.sync.dma_start(out=w[:], in_=W_d[l][:, :])
                w_sb[l] = w
            b_sb = {}
            for l in range(4):
                b = const.tile([P, 1], F32, name=f"b{l}sb")
                nc.sync.dma_start(out=b[:], in_=b_d[l][:, :])
                b_sb[l] = b
            gmin = int(os.environ.get("GCN_MIN", "0"))
            iota_sb = const.tile([P, P], F32, name="iotasb")
            nc.sync.dma_start(out=iota_sb[:], in_=iota_d[:, :])
            iden_sb = const.tile([P, P], F32, name="idensb")
            iden16_sb = const.tile([P, P], F16, name="iden16sb")
            if not gmin:
                make_identity(nc, iden_sb[:])
                make_identity(nc, iden16_sb[:])
            srcidx_sb = const.tile([P, NCH * P // 16], I16, name="srcidxsb")
            nc.sync.dma_start(out=srcidx_sb[:], in_=srcidx_d[:, :])
            dstid_sb = const.tile([P, NCH], F32, name="dstidsb")
            disrep_sb = const.tile([P, N_loc], F16, name="disrepsb")
            poolid_sb = const.tile([P, T * GW], F32, name="poolidsb")
            if not gmin:
                nc.sync.dma_start(out=dstid_sb[:], in_=dstid_d[:, :])
                nc.sync.dma_start(out=disrep_sb[:], in_=disrep_d[:, :])
                nc.sync.dma_start(out=poolid_sb[:], in_=poolid_d[:, :])

            H = hpool.tile([P, N_loc], F32, name="H", tag="ha")

            # Join all const-load DMA sems into the DVE engine clock so later
            # DVE tensor_tensor ops (2 sync-wait slots in the ISA) don't have
            # to carry per-DMA waits themselves.
            if not gmin:
                joiner = const.tile([P, 1], F32, name="joiner")
                for cst in [iota_sb, dstid_sb, disrep_sb, poolid_sb,
                            b_sb[0], b_sb[1], b_sb[2], b_sb[3]]:
                    nc.vector.tensor_copy(out=joiner[:, :1], in_=cst[:, :1])

            # ---- phase 1: M^T = W^T @ Hprev^T, scale by dis, transpose,
            #      write node-major Hs to local DRAM (skipped for l=0/emb).
            def phase1(l, HprevT, HsLocal):
                dout = DOUT.get(l, D)
                nk = (N_loc + 511) // 512
                for k in range(nk):
                    c0 = k * 512
                    cw = min(512, N_loc - c0)
                    if l == 0:
                        # stream x^T chunks from DRAM (saves 6.5MB SBUF)
                        xst = stage.tile([P, 512], F32, name="xst", tag="ms")
                        nc.sync.dma_start(out=xst[:, :cw],
                                          in_=xT_d[:, c0:c0 + cw])
                        rhs_ap = xst[:, :cw]
                    else:
                        rhs_ap = HprevT[:, c0:c0 + cw]
                    mm = pm.tile([P, 512], F32, name="mm", tag="pm")
                    nc.tensor.matmul(mm[:dout, :cw], lhsT=w_sb[l][:, :dout],
                                     rhs=rhs_ap,
                                     start=True, stop=True)
                    if l == 0:
                        nc.vector.tensor_scalar(
                            out=H[:, c0:c0 + cw], in0=mm[:, :cw],
                            scalar1=b_sb[0][:, :], scalar2=None, op0=OP.add)
                        continue
                    ms = stage.tile([P, 512], F16, name="ms", tag="ms")
                    nc.vector.tensor_tensor(
                        out=ms[:dout, :cw], in0=mm[:dout, :cw],
                        in1=disrep_sb[:dout, c0:c0 + cw], op=OP.mult)
                    for tt in range(cw // P):
                        tglob = k * 4 + tt
                        ptt = pt.tile([P, P], F16, name="ptt", tag="pt")
                        nc.tensor.transpose(
                            out=ptt[:, :dout],
                            in_=ms[:dout, tt * P:(tt + 1) * P],
                            identity=iden16_sb[:dout, :dout])
                        hs = stage.tile([P, P], F16, name="hs", tag="hs")
                        nc.vector.tensor_copy(out=hs[:, :dout],
                                              in_=ptt[:, :dout])
                        nc.sync.dma_start(
                            out=HsLocal[tglob * P:(tglob + 1) * P, :],
                            in_=hs[:, :dout])

            # ---- phase 2: gather + scatter-add into local dst tiles.
            def phase2(l, HsFull, H3):
                dout = DOUT[l]
                NB = (T + TB - 1) // TB
                for tb in range(NB):
                    tbg = min(TB, T - tb * TB)
                    block_base = tb * NQ * TB * CPS
                    num = tbg * CPS * P
                    Rq = []
                    for q in range(NQ):
                        R = rpool.tile([P, TB * CPS * dout], F16,
                                       name="R", tag="R")
                        c0 = (block_base + q * tbg * CPS) * P // 16
                        gi = phase2.gcount = getattr(phase2, "gcount", 0) + 1
                        real_n = int(os.environ.get("GCN_REAL_GATHERS",
                                                    "99999"))
                        if (int(os.environ.get("GCN_FAKE_GATHER", "0"))
                                or gi > real_n):
                            nc.sync.dma_start(
                                out=R[:, :tbg * CPS * dout],
                                in_=HsFull[q * QR:
                                           q * QR + P * tbg * CPS,
                                           :].rearrange(
                                    "(p c) e -> p (c e)", p=P))
                        else:
                            nc.gpsimd.dma_gather(
                                out_ap=R[:, :tbg * CPS * dout].rearrange(
                                    "p (c e) -> p c e", e=dout),
                                in_ap=HsFull[q * QR:(q + 1) * QR, :],
                                idxs_ap=srcidx_sb[:, c0:c0 + num // 16],
                                num_idxs=num,
                                num_idxs_reg=num,
                                elem_size=dout,
                                single_packet=(num <= 1008),
                                queue_num=q % 4)
                        Rq.append(R)
                    if int(os.environ.get("GCN_NO_AGG", "0")):
                        for q in range(NQ):
                            nc.vector.tensor_copy(
                                out=H[:, (tb * NQ + q) % N_loc:
                                      (tb * NQ + q) % N_loc + 1],
                                in_=Rq[q][:, 0:1])
                        continue
                    for tl in range(tbg):
                        phase2_tile(l, tb * TB + tl, tl, Rq, H3)

            def phase2_tile(l, t, tl, Rq, H3):
                dout = DOUT[l]
                nch = NQ * CPS
                S = spool.tile([P, NQ * CPS * P], F16, name="S", tag="S")
                nc.vector.tensor_tensor(
                    out=S[:].rearrange("p (c d) -> p c d", d=P),
                    in0=dstid_sb[:, t * nch:(t + 1) * nch]
                        .unsqueeze(2).broadcast_to([P, nch, P]),
                    in1=iota_sb[:, :].unsqueeze(1).broadcast_to([P, nch, P]),
                    op=OP.is_equal)
                agg = pa.tile([P, P], F32, name="agg", tag="pa")
                for q in range(NQ):
                    for c in range(CPS):
                        k = q * CPS + c
                        nc.tensor.matmul(
                            agg[:dout, :],
                            lhsT=Rq[q][:, (tl * CPS + c) * dout:
                                       (tl * CPS + c + 1) * dout],
                            rhs=S[:, k * P:(k + 1) * P],
                            start=(k == 0), stop=(k == nch - 1))
                tmp = stage.tile([P, P], F32, name="tmp", tag="tmp")
                nc.vector.tensor_tensor(
                    out=tmp[:dout, :], in0=agg[:dout, :],
                    in1=disrep_sb[:dout, t * P:(t + 1) * P], op=OP.mult)
                if l < 3:
                    nc.vector.tensor_scalar(
                        out=H[:, t * P:(t + 1) * P], in0=tmp[:, :],
                        scalar1=b_sb[l][:, :], scalar2=0.0,
                        op0=OP.add, op1=OP.max)
                else:
                    t2 = stage.tile([P, P], F32, name="t2", tag="tmp")
                    nc.vector.tensor_scalar(
                        out=t2[:dout, :], in0=tmp[:dout, :],
                        scalar1=b_sb[3][:dout, :], scalar2=None,
                        op0=OP.add)
                    ptt = pt.tile([P, P], F32, name="ptt2", tag="pt")
                    nc.tensor.transpose(
                        out=ptt[:, :dout], in_=t2[:dout, :],
                        identity=iden_sb[:dout, :dout])
                    nc.vector.tensor_copy(
                        out=H3[:, t * DO:(t + 1) * DO],
                        in_=ptt[:, :DO])

            def dump_dbg(buf, width=None):
                nc.sync.dma_start(out=dbg_d[:, :width] if width else dbg_d[:, :],
                                  in_=buf[:, :width] if width else buf[:, :])

            # ---- the network
            max_layers = int(os.environ.get("GCN_MAX_LAYERS", "3"))
            skip_p1 = int(os.environ.get("GCN_SKIP_P1", "0"))
            if not skip_p1:
                phase1(0, None, None)  # embedding -> H (streams xT from DRAM)
            if dbg_stage == "h0":
                dump_dbg(H)
            H3 = None
            for l in range(1, max_layers + 1):
                dout = DOUT[l]
                HsLocal = dram.tile([N_loc, dout], F16, name=f"hsl{l}",
                                    tag="hsl")
                if skip_p1:
                    for r in range(T):
                        hz = stage.tile([P, P], F16, name="hz", tag="hs")
                        nc.vector.memset(hz[:, :dout], 1.0)
                        nc.sync.dma_start(
                            out=HsLocal[r * P:(r + 1) * P, :],
                            in_=hz[:, :dout])
                else:
                    phase1(l, H, HsLocal)
                HsFull = dram.tile(
                    [n_cores * N_loc, dout], F16, name=f"hsf{l}", tag="hsf",
                    addr_space="Local"
                    if int(os.environ.get("GCN_LOCAL_HSF", "0"))
                    else "Shared")
                if n_cores > 1 and not int(os.environ.get("GCN_NO_CC", "0")):
                    nc.gpsimd.collective_compute(
                        "AllGather", OP.bypass,
                        replica_groups=[list(range(n_cores))],
                        ins=[HsLocal[:, :].opt()],
                        outs=[HsFull[:, :].opt()])
                else:
                    for cc_i in range(n_cores):
                        nc.sync.dma_start(
                            out=HsFull[cc_i * N_loc:(cc_i + 1) * N_loc, :],
                            in_=HsLocal[:, :])
                if dbg_stage == f"hsf{l}":
                    nc.sync.dma_start(out=dbg2_d[:, :], in_=HsFull[:, :])
                if l == 3:
                    H3 = hpool.tile([P, T * DO], F32, name="H3", tag="hx")
                phase2(l, HsFull, H3)
                if dbg_stage == f"h{l}":
                    dump_dbg(H if l < 3 else H3, None if l < 3 else T * DO)

            # ---- global add pool
            if max_layers < 3:
                zz = stage.tile([P, DO], F32, name="zz", tag="ost")
                nc.gpsimd.memset(zz[:], 0.0)
                for w in range(GW):
                    nc.sync.dma_start(out=out_d[w * P:(w + 1) * P, :],
                                      in_=zz[:])
                return nc
            for w in range(GW):
                pp = pt.tile([P, DO], F32, name="pp", tag="pp")
                for t in range(T):
                    sp = spool.tile([P, P], F32, name="sp", tag="sp")
                    nc.vector.tensor_tensor(
                        out=sp[:],
                        in0=poolid_sb[:, w * T + t:w * T + t + 1]
                            .to_broadcast([P, P]),
                        in1=iota_sb[:, :], op=OP.is_equal)
                    nc.tensor.matmul(pp[:], lhsT=sp[:],
                                     rhs=H3[:, t * DO:(t + 1) * DO],
                                     start=(t == 0), stop=(t == T - 1))
                ost = stage.tile([P, DO], F32, name="ost", tag="ost")
                nc.vector.tensor_copy(out=ost[:], in_=pp[:])
                nc.sync.dma_start(out=out_d[w * P:(w + 1) * P, :],
                                  in_=ost[:])

    return nc


# ----------------------------------------------------------------------------
# Driver
# ----------------------------------------------------------------------------

def _run(x, edge_index, batch, W_emb, b_emb, W1, b1, W2, b2, W3, b3,
         G=G_TOTAL, n_cores=N_CORES, trace=False):
    x = np.ascontiguousarray(np.asarray(x, dtype=np.float32))
    edge_index = np.ascontiguousarray(np.asarray(edge_index, dtype=np.int64))
    batch_np = np.ascontiguousarray(np.asarray(batch, dtype=np.int64))

    cfg, in_maps, bounds, g_lo, g_cnt = _preprocess(
        x, edge_index, batch_np, n_cores, G)

    def bpad(b):
        v = np.zeros((P, 1), dtype=np.float32)
        b = np.asarray(b, dtype=np.float32).reshape(-1)
        v[:b.shape[0], 0] = b
        return v

    W3p = np.zeros((128, 128), dtype=np.float32)
    W3p[:, :np.asarray(W3).shape[1]] = np.asarray(W3, dtype=np.float32)
    shared = dict(
        W0=np.asarray(W_emb, dtype=np.float32),
        W1=np.asarray(W1, dtype=np.float32),
        W2=np.asarray(W2, dtype=np.float32),
        W3=W3p,
        b0=bpad(b_emb), b1=bpad(b1), b2=bpad(b2), b3=bpad(b3))
    for m in in_maps:
        m.update(shared)

    nc = _build_program(cfg)
    nc.finalize()
    res = run_bass_kernel_spmd(nc, in_maps, list(range(n_cores)),
                               trace=trace)

    out = np.zeros((G, 64), dtype=np.float32)
    for c in range(n_cores):
        oc = np.asarray(res.results[c]["out"])
        if g_cnt[c] > 0:
            out[g_lo[c]:g_lo[c] + g_cnt[c]] = oc[:g_cnt[c]]
    return out, res


def kernel(**inputs):
    out, _ = _run(G=G_TOTAL, n_cores=N_CORES,
                  trace=bool(int(os.environ.get("GCN_TRACE", "0"))),
                  **inputs)
    return out



# revision 20
# speedup vs baseline: 3.5405x; 1.4270x over previous
"""Trainium2 Bass kernel for a 3-layer GCN (Kipf-Welling, symmetric norm,
self-loops) with global add pooling.

Distribution: nodes (graph-aligned contiguous ranges) are sharded across 8
NeuronCores.  Each core owns the aggregation (scatter-add) for its local dst
nodes.  Per layer the (dis-scaled) projected features Hs are exchanged in 4
tile-BANDS via 4 pipelined AllGathers; band j's edge gathers run on SWDGE
queue j so desc-gen pipelines across the 4 Q7 cpu pairs.

Math (matches the jax reference exactly):
    deg  = indeg + 1, dis = deg^-1/2
    Hsf  = dis * (H @ W)                      (feature-major, kept in SBUF)
    agg_d = dis_d * (sum_{e:(s->d)} Hsf_s  +  Hsf_d) + b     (self loop local)
    H'   = relu(agg)                          (no relu on layer 3)
    out  = segment_sum(H3, batch)

The edge scatter-add is a matmul with an on-the-fly selection matrix
S[e, d] = (dst_id[e] == d) in fp16; gathered source rows are fp16.
Layer 3 is zero-padded to 128 output features so fp16 table rows stay 256B
(dma_gather elem_size restriction).
"""

import os
import sys
import math

import numpy as np

sys.path.insert(0, "/opt/trn_rl_repo")

import concourse.bass as bass  # noqa: E402
import concourse.bacc as bacc  # noqa: E402
import concourse.tile as tile  # noqa: E402
from concourse import mybir  # noqa: E402
from concourse.bass_utils import run_bass_kernel_spmd  # noqa: E402
from concourse.masks import make_identity  # noqa: E402

P = 128
F32 = mybir.dt.float32
F16 = mybir.dt.float16
I32 = mybir.dt.int32
I16 = mybir.dt.int16
OP = mybir.AluOpType

N_CORES = 8
G_TOTAL = 1000  # graphs in the batch (fixed by the problem)
NB_BANDS = 4    # src-tile bands (= SWDGE queues = pipelined allgathers)
TB = 2          # dst tiles per gather block


# ----------------------------------------------------------------------------
# Host-side preprocessing: shard nodes/edges, build gather/selection metadata.
# ----------------------------------------------------------------------------

def _preprocess(x, edge_index, batch, n_cores, G):
    N = x.shape[0]
    src = edge_index[0].astype(np.int64)
    dst = edge_index[1].astype(np.int64)
    batch = batch.astype(np.int64)

    # graph-aligned shard boundaries near equal node counts
    graph_start = np.searchsorted(batch, np.arange(G + 1))  # [G+1], node idx
    bounds = [0]
    for c in range(1, n_cores):
        target = (c * N) // n_cores
        gi = np.searchsorted(graph_start, target)
        lo = graph_start[gi - 1] if gi > 0 else 0
        hi = graph_start[gi] if gi <= G else N
        b = int(hi if (hi - target) <= (target - lo) else lo)
        b = max(b, bounds[-1])  # keep non-decreasing
        bounds.append(b)
    bounds.append(N)
    bounds = np.asarray(bounds, dtype=np.int64)

    shard_sizes = bounds[1:] - bounds[:-1]
    N_loc = int(math.ceil(int(shard_sizes.max()) / P) * P)
    T = N_loc // P
    NB = (T + TB - 1) // TB

    # band boundaries over local tiles
    bs = (T + NB_BANDS - 1) // NB_BANDS
    band_lo = [min(j * bs, T) for j in range(NB_BANDS + 1)]
    band_tiles = [band_lo[j + 1] - band_lo[j] for j in range(NB_BANDS)]

    # normalization (index-derived scalars)
    deg = np.bincount(dst, minlength=N).astype(np.float32) + np.float32(1.0)
    dis = (np.float32(1.0) / np.sqrt(deg)).astype(np.float32)

    # src row id within its band table:
    # band table j rows: (src_core * band_tiles[j] + tile_in_band)*128 + pos
    core_of = np.searchsorted(bounds, dst, side="right") - 1
    core_of_src = np.searchsorted(bounds, src, side="right") - 1
    src_loc = src - bounds[core_of_src]
    src_tile = src_loc // P
    src_band = np.minimum(src_tile // bs, NB_BANDS - 1)
    src_row = (core_of_src * np.asarray(band_tiles)[src_band]
               + (src_tile - np.asarray(band_lo)[src_band])) * P + src_loc % P
    assert int(src_row.max(initial=0)) < 32768

    # ---- per-core edge bucketing by (dst block, src band), sorted by dst
    per_core = []
    for c in range(n_cores):
        m = core_of == c
        dstl = dst[m] - bounds[c]
        rows = src_row[m]
        bands = src_band[m]
        blk = dstl // (TB * P)
        key = blk * NB_BANDS + bands
        order = np.lexsort((dstl, key))
        dstl, rows, key = dstl[order], rows[order], key[order]
        cnt = np.bincount(key, minlength=NB * NB_BANDS)
        per_core.append((dstl, rows, key, cnt))

    cnts = np.stack([pc[3] for pc in per_core])        # [cores, NB*NB_BANDS]
    CH = np.ceil(cnts.max(axis=0) / P).astype(np.int64)  # chunks per group
    ch_off = np.concatenate([[0], np.cumsum(CH)])      # chunk col offsets
    total_ch = int(ch_off[-1])
    # idx16 column offset per group (16-wrapped, so 8 cols per chunk)
    col_off = ch_off * (P // 16)

    # per-(group, tile-in-block) chunk subranges, unioned over cores
    rng_lo = np.full((NB * NB_BANDS, TB), 10 ** 9, dtype=np.int64)
    rng_hi = np.full((NB * NB_BANDS, TB), -1, dtype=np.int64)

    in_maps = []
    g_lo = []
    g_cnt = []
    for c in range(n_cores):
        dstl, rows, key, cnt = per_core[c]
        goff = np.concatenate([[0], np.cumsum(cnt)])[:-1]
        slot = (np.arange(dstl.shape[0]) - goff[key]) + ch_off[key] * P

        # pad slots gather row 0 (valid) — trailing -1 trim desyncs the DGE
        # ring bookkeeping (decode reserves by untrimmed count); dstid 30000
        # keeps padded slots out of every S matrix.
        idx16 = np.zeros((16, total_ch * P // 16), dtype=np.int16)
        idx16[slot % 16, slot // 16] = rows.astype(np.int16)
        srcidx = np.tile(idx16, (8, 1))

        dstid = np.full((P, total_ch), 30000.0, dtype=np.float16)
        dstid[slot % P, slot // P] = (dstl % (TB * P)).astype(np.float32)

        # per-core tile chunk ranges -> union
        tl = (dstl // P) % TB
        gkey = key * TB + tl
        gcnt = np.bincount(gkey, minlength=NB * NB_BANDS * TB)
        goff2 = np.concatenate([[0], np.cumsum(gcnt)])
        for g in range(NB * NB_BANDS):
            for t2 in range(TB):
                s0, s1 = goff2[g * TB + t2], goff2[g * TB + t2 + 1]
                if s1 > s0:
                    a = (s0 - goff[g]) // P
                    b = (s1 - 1 - goff[g]) // P + 1
                    rng_lo[g, t2] = min(rng_lo[g, t2], a)
                    rng_hi[g, t2] = max(rng_hi[g, t2], b)

        n_real = int(bounds[c + 1] - bounds[c])
        dis_loc = np.ones(N_loc, dtype=np.float32)
        dis_loc[:n_real] = dis[bounds[c]:bounds[c + 1]]
        disrep = np.broadcast_to(dis_loc.astype(np.float16), (P, N_loc)).copy()

        xT = np.zeros((P, N_loc), dtype=np.float32)
        xT[:, :n_real] = x[bounds[c]:bounds[c + 1]].T

        bloc = batch[bounds[c]:bounds[c + 1]]
        glo = int(bloc[0]) if n_real > 0 else 0
        gct = int(bloc[-1]) + 1 - glo if n_real > 0 else 0
        g_lo.append(glo)
        g_cnt.append(gct)
        in_maps.append(dict(srcidx=srcidx, dstid=dstid, disrep=disrep, xT=xT,
                            _bloc=bloc - glo, _n_real=n_real))

    GW = max(1, int(math.ceil(max(g_cnt) / P)))
    iota2 = np.zeros((P, TB * P), dtype=np.float16)
    for t2 in range(TB):
        iota2[:, t2 * P:(t2 + 1) * P] = np.arange(P) + t2 * P
    for c in range(n_cores):
        d = in_maps[c]
        bloc, n_real = d.pop("_bloc"), d.pop("_n_real")
        poolid = np.full((P, T * GW), 30000.0, dtype=np.float16)
        j = np.arange(n_real)
        for w in range(GW):
            poolid[j % P, (j // P) + w * T] = (bloc - w * P).astype(np.float32)
        d["poolid"] = poolid
        d["iota2"] = iota2

    rngs = np.where(rng_hi < 0, 0, rng_hi - np.maximum(rng_lo, 0))
    cfg = dict(T=T, NB=NB, GW=GW, N_loc=N_loc, n_cores=n_cores,
               band_tiles=band_tiles, band_lo=band_lo,
               CH=CH.tolist(), ch_off=ch_off.tolist(),
               col_off=col_off.tolist(),
               rng_lo=np.maximum(rng_lo, 0).tolist(),
               rng_hi=np.maximum(rng_hi, 0).tolist(),
               max_rng=int(rngs.max()), max_ch=int(CH.max()))
    return cfg, in_maps, bounds, g_lo, g_cnt


# ----------------------------------------------------------------------------
# Bass program
# ----------------------------------------------------------------------------

def _build_program(cfg):
    T, NB, GW, N_loc = cfg["T"], cfg["NB"], cfg["GW"], cfg["N_loc"]
    n_cores = cfg["n_cores"]
    band_tiles, band_lo = cfg["band_tiles"], cfg["band_lo"]
    CH, ch_off, col_off = cfg["CH"], cfg["ch_off"], cfg["col_off"]
    rng_lo, rng_hi = cfg["rng_lo"], cfg["rng_hi"]
    total_ch = ch_off[-1]
    D, DO = 128, 64

    nc = bacc.Bacc(None, num_devices=n_cores, num_swdge_queues=4)

    xT_d = nc.dram_tensor("xT", [P, N_loc], F32, kind="ExternalInput")
    W_d = {l: nc.dram_tensor(f"W{l}", [D, D], F32, kind="ExternalInput")
           for l in range(4)}
    b_d = {l: nc.dram_tensor(f"b{l}", [P, 1], F32, kind="ExternalInput")
           for l in range(4)}
    srcidx_d = nc.dram_tensor("srcidx", [P, total_ch * P // 16], I16,
                              kind="ExternalInput")
    dstid_d = nc.dram_tensor("dstid", [P, total_ch], F16,
                             kind="ExternalInput")
    disrep_d = nc.dram_tensor("disrep", [P, N_loc], F16, kind="ExternalInput")
    poolid_d = nc.dram_tensor("poolid", [P, T * GW], F16, kind="ExternalInput")
    iota2_d = nc.dram_tensor("iota2", [P, TB * P], F16, kind="ExternalInput")
    out_d = nc.dram_tensor("out", [GW * P, DO], F32, kind="ExternalOutput")
    dbg_stage = os.environ.get("GCN_DBG_STAGE", "")
    dbg_d = None
    if dbg_stage.startswith("h"):
        dbg_d = nc.dram_tensor("dbg", [P, N_loc], F32, kind="ExternalOutput")

    with tile.TileContext(nc) as tc:
        with tc.tile_pool(name="const", bufs=1) as const, \
             tc.tile_pool(name="hpool", bufs=1) as hpool, \
             tc.tile_pool(name="stage", bufs=3) as stage, \
             tc.tile_pool(name="rpool", bufs=2 * NB_BANDS) as rpool, \
             tc.tile_pool(name="spool", bufs=2) as spool, \
             tc.tile_pool(name="dram", bufs=2, space="DRAM") as dram, \
             tc.tile_pool(name="pm", bufs=2, space="PSUM") as pm, \
             tc.tile_pool(name="pt", bufs=2, space="PSUM") as pt, \
             tc.tile_pool(name="pa", bufs=2, space="PSUM") as pa:

            # ---- constants into SBUF
            w_sb = {}
            for l in range(4):
                w = const.tile([D, D], F32, name=f"w{l}sb")
                nc.sync.dma_start(out=w[:], in_=W_d[l][:, :])
                w_sb[l] = w
            b_sb = {}
            for l in range(4):
                b = const.tile([P, 1], F32, name=f"b{l}sb")
                nc.sync.dma_start(out=b[:], in_=b_d[l][:, :])
                b_sb[l] = b
            iota2_sb = const.tile([P, TB * P], F16, name="iota2sb")
            nc.sync.dma_start(out=iota2_sb[:], in_=iota2_d[:, :])
            iden_sb = const.tile([P, P], F32, name="idensb")
            make_identity(nc, iden_sb[:])
            iden16_sb = const.tile([P, P], F16, name="iden16sb")
            make_identity(nc, iden16_sb[:])
            srcidx_sb = const.tile([P, total_ch * P // 16], I16,
                                   name="srcidxsb")
            nc.sync.dma_start(out=srcidx_sb[:], in_=srcidx_d[:, :])
            dstid_sb = const.tile([P, total_ch], F16, name="dstidsb")
            disrep_sb = const.tile([P, N_loc], F16, name="disrepsb")
            poolid_sb = const.tile([P, T * GW], F16, name="poolidsb")
            nc.sync.dma_start(out=dstid_sb[:], in_=dstid_d[:, :])
            nc.sync.dma_start(out=disrep_sb[:], in_=disrep_d[:, :])
            nc.sync.dma_start(out=poolid_sb[:], in_=poolid_d[:, :])

            H = hpool.tile([P, N_loc], F32, name="H", tag="ha")
            Hsf = hpool.tile([P, N_loc], F16, name="Hsf", tag="hsf")

            # Zero the R pool once: trimmed (-1) gather slots are never
            # written by the DMA, and a NaN in untouched SBUF would poison
            # the scatter matmul (0 * NaN).  After this, stale slots only
            # ever hold old finite Hs values.
            for _ in range(2 * NB_BANDS):
                R0 = rpool.tile([P, cfg["max_ch"] * D], F16,
                                name="R", tag="R")
                nc.vector.memset(R0[:, :], 0.0)

            # Join const-load DMA sems into the DVE engine clock so later DVE
            # tensor_tensor ops don't carry per-DMA waits themselves.
            joiner = const.tile([P, 1], F32, name="joiner")
            for cst in [iota2_sb, dstid_sb, disrep_sb, poolid_sb,
                        b_sb[0], b_sb[1], b_sb[2], b_sb[3]]:
                nc.vector.tensor_copy(out=joiner[:, :1], in_=cst[:, :1])

            # ---- phase 1: M^T = W^T @ Hprev^T; l=0 -> H, l>=1 -> Hsf (scaled
            #      by dis), then transpose tiles and write fp16 band tables.
            def phase1(l, HsBands):
                nk = (N_loc + 511) // 512
                for k in range(nk):
                    c0 = k * 512
                    cw = min(512, N_loc - c0)
                    if l == 0:
                        xst = stage.tile([P, 512], F32, name="xst", tag="ms")
                        nc.sync.dma_start(out=xst[:, :cw],
                                          in_=xT_d[:, c0:c0 + cw])
                        rhs_ap = xst[:, :cw]
                    else:
                        rhs_ap = H[:, c0:c0 + cw]
                    mm = pm.tile([P, 512], F32, name="mm", tag="pm")
                    nc.tensor.matmul(mm[:, :cw], lhsT=w_sb[l][:, :],
                                     rhs=rhs_ap, start=True, stop=True)
                    if l == 0:
                        nc.vector.tensor_scalar(
                            out=H[:, c0:c0 + cw], in0=mm[:, :cw],
                            scalar1=b_sb[0][:, :], scalar2=None, op0=OP.add)
                        continue
                    nc.vector.tensor_tensor(
                        out=Hsf[:, c0:c0 + cw], in0=mm[:, :cw],
                        in1=disrep_sb[:, c0:c0 + cw], op=OP.mult)
                    for tt in range(cw // P):
                        t = k * 4 + tt
                        j = 0
                        while t >= band_lo[j + 1]:
                            j += 1
                        trow = (t - band_lo[j]) * P
                        ptt = pt.tile([P, P], F16, name="ptt", tag="pt")
                        nc.tensor.transpose(
                            out=ptt[:, :],
                            in_=Hsf[:, t * P:(t + 1) * P],
                            identity=iden16_sb[:, :])
                        hs = stage.tile([P, P], F16, name="hs", tag="hs")
                        nc.vector.tensor_copy(out=hs[:, :], in_=ptt[:, :])
                        nc.sync.dma_start(
                            out=HsBands[j][trow:trow + P, :],
                            in_=hs[:, :])

            # ---- phase 2: per block, gather 4 band groups (queues 0-3),
            #      then per tile scatter-matmul + local self term.
            def phase2(l, HsFullBands, H3):
                for tb in range(NB):
                    tbg = min(TB, T - tb * TB)
                    Rb = []
                    for j in range(NB_BANDS):
                        g = tb * NB_BANDS + j
                        nch = CH[g]
                        if nch == 0:
                            Rb.append(None)
                            continue
                        R = rpool.tile([P, cfg["max_ch"] * D], F16,
                                       name="R", tag="R")
                        num = nch * P
                        nc.gpsimd.dma_gather(
                            out_ap=R[:, :nch * D].rearrange(
                                "p (c e) -> p c e", e=D),
                            in_ap=HsFullBands[j][:, :],
                            idxs_ap=srcidx_sb[:, col_off[g]:
                                              col_off[g] + num // 16],
                            num_idxs=num,
                            num_idxs_reg=num,
                            elem_size=D,
                            single_packet=False,
                            queue_num=j)
                        Rb.append(R)
                    for tl in range(tbg):
                        phase2_tile(l, tb, tl, Rb, H3)

            def phase2_tile(l, tb, tl, Rb, H3):
                t = tb * TB + tl
                # selection matrices + accumulate matmuls over 4 bands
                mms = []
                for j in range(NB_BANDS):
                    g = tb * NB_BANDS + j
                    a, b2 = rng_lo[g][tl], rng_hi[g][tl]
                    if b2 <= a or Rb[j] is None:
                        continue
                    nr = b2 - a
                    S = spool.tile([P, cfg["max_rng"] * P], F16,
                                   name="S", tag="S")
                    nc.vector.tensor_tensor(
                        out=S[:, :nr * P].rearrange("p (c d) -> p c d", d=P),
                        in0=dstid_sb[:, ch_off[g] + a:ch_off[g] + b2]
                            .unsqueeze(2).broadcast_to([P, nr, P]),
                        in1=iota2_sb[:, tl * P:(tl + 1) * P]
                            .unsqueeze(1).broadcast_to([P, nr, P]),
                        op=OP.is_equal)
                    for cc in range(a, b2):
                        mms.append((Rb[j], cc, S, cc - a))
                agg = pa.tile([P, P], F32, name="agg", tag="pa")
                if not mms:
                    nc.vector.memset(agg[:, :], 0.0)
                for k, (R, cc, S, sc) in enumerate(mms):
                    nc.tensor.matmul(
                        agg[:, :],
                        lhsT=R[:, cc * D:(cc + 1) * D],
                        rhs=S[:, sc * P:(sc + 1) * P],
                        start=(k == 0), stop=(k == len(mms) - 1))
                # out = dis_d * (agg + Hsf_d) + b ; relu except l=3
                tmp = stage.tile([P, P], F32, name="tmp", tag="tmp")
                nc.vector.tensor_tensor(
                    out=tmp[:, :], in0=agg[:, :],
                    in1=Hsf[:, t * P:(t + 1) * P], op=OP.add)
                nc.vector.tensor_tensor(
                    out=tmp[:, :], in0=tmp[:, :],
                    in1=disrep_sb[:, t * P:(t + 1) * P], op=OP.mult)
                if l < 3:
                    nc.vector.tensor_scalar(
                        out=H[:, t * P:(t + 1) * P], in0=tmp[:, :],
                        scalar1=b_sb[l][:, :], scalar2=0.0,
                        op0=OP.add, op1=OP.max)
                else:
                    t2 = stage.tile([P, P], F32, name="t2", tag="tmp2")
                    nc.vector.tensor_scalar(
                        out=t2[:, :], in0=tmp[:, :],
                        scalar1=b_sb[3][:, :], scalar2=None, op0=OP.add)
                    ptt = pt.tile([P, P], F32, name="ptt2", tag="pt")
                    nc.tensor.transpose(
                        out=ptt[:, :], in_=t2[:, :],
                        identity=iden_sb[:, :])
                    nc.vector.tensor_copy(
                        out=H3[:, t * DO:(t + 1) * DO],
                        in_=ptt[:, :DO])

            def dump_dbg(buf, width=None):
                nc.sync.dma_start(
                    out=dbg_d[:, :width] if width else dbg_d[:, :],
                    in_=buf[:, :width] if width else buf[:, :])

            # ---- the network
            max_layers = int(os.environ.get("GCN_MAX_LAYERS", "3"))
            phase1(0, None)  # embedding -> H (streams xT from DRAM)
            if dbg_stage == "h0":
                dump_dbg(H)
            H3 = None
            for l in range(1, max_layers + 1):
                HsBands = [dram.tile([band_tiles[j] * P, D], F16,
                                     name=f"hsl{l}_{j}", tag=f"hsl{j}")
                           for j in range(NB_BANDS)]
                phase1(l, HsBands)
                HsFullBands = []
                no_cc = int(os.environ.get("GCN_NO_CC", "0"))
                for j in range(NB_BANDS):
                    HsFull = dram.tile([n_cores * band_tiles[j] * P, D], F16,
                                       name=f"hsf{l}_{j}", tag=f"hsf{j}",
                                       addr_space="Local" if no_cc
                                       else "Shared")
                    if no_cc:
                        nb_rows = band_tiles[j] * P
                        for cc_i in range(n_cores):
                            nc.sync.dma_start(
                                out=HsFull[cc_i * nb_rows:
                                           (cc_i + 1) * nb_rows, :],
                                in_=HsBands[j][:, :])
                    else:
                        nc.gpsimd.collective_compute(
                            "AllGather", OP.bypass,
                            replica_groups=[list(range(n_cores))],
                            ins=[HsBands[j][:, :].opt()],
                            outs=[HsFull[:, :].opt()])
                    HsFullBands.append(HsFull)
                if l == 3:
                    H3 = hpool.tile([P, T * DO], F16, name="H3", tag="hx")
                phase2(l, HsFullBands, H3)
                if dbg_stage == f"h{l}":
                    dump_dbg(H if l < 3 else H3, None if l < 3 else T * DO)

            # ---- global add pool
            for w in range(GW):
                pp = pt.tile([P, DO], F32, name="pp", tag="pp")
                for t in range(T):
                    sp = spool.tile([P, P], F16, name="sp", tag="sp")
                    nc.vector.tensor_tensor(
                        out=sp[:],
                        in0=poolid_sb[:, w * T + t:w * T + t + 1]
                            .to_broadcast([P, P]),
                        in1=iota2_sb[:, :P], op=OP.is_equal)
                    nc.tensor.matmul(pp[:], lhsT=sp[:],
                                     rhs=H3[:, t * DO:(t + 1) * DO],
                                     start=(t == 0), stop=(t == T - 1))
                ost = stage.tile([P, DO], F32, name="ost", tag="ost")
                nc.vector.tensor_copy(out=ost[:], in_=pp[:])
                nc.sync.dma_start(out=out_d[w * P:(w + 1) * P, :],
                                  in_=ost[:])

    return nc


# ----------------------------------------------------------------------------
# Driver
# ----------------------------------------------------------------------------

def _run(x, edge_index, batch, W_emb, b_emb, W1, b1, W2, b2, W3, b3,
         G=G_TOTAL, n_cores=N_CORES, trace=False):
    x = np.ascontiguousarray(np.asarray(x, dtype=np.float32))
    edge_index = np.ascontiguousarray(np.asarray(edge_index, dtype=np.int64))
    batch_np = np.ascontiguousarray(np.asarray(batch, dtype=np.int64))

    cfg, in_maps, bounds, g_lo, g_cnt = _preprocess(
        x, edge_index, batch_np, n_cores, G)

    def bpad(b):
        v = np.zeros((P, 1), dtype=np.float32)
        b = np.asarray(b, dtype=np.float32).reshape(-1)
        v[:b.shape[0], 0] = b
        return v

    W3p = np.zeros((128, 128), dtype=np.float32)
    W3p[:, :np.asarray(W3).shape[1]] = np.asarray(W3, dtype=np.float32)
    shared = dict(
        W0=np.asarray(W_emb, dtype=np.float32),
        W1=np.asarray(W1, dtype=np.float32),
        W2=np.asarray(W2, dtype=np.float32),
        W3=W3p,
        b0=bpad(b_emb), b1=bpad(b1), b2=bpad(b2), b3=bpad(b3))
    for m in in_maps:
        m.update(shared)

    nc = _build_program(cfg)
    nc.finalize()
    res = run_bass_kernel_spmd(nc, in_maps, list(range(n_cores)),
                               trace=trace)

    out = np.zeros((G, 64), dtype=np.float32)
    for c in range(n_cores):
        oc = np.asarray(res.results[c]["out"])
        if g_cnt[c] > 0:
            out[g_lo[c]:g_lo[c] + g_cnt[c]] = oc[:g_cnt[c]]
    return out, res


def kernel(**inputs):
    out, _ = _run(G=G_TOTAL, n_cores=N_CORES,
                  trace=bool(int(os.environ.get("GCN_TRACE", "0"))),
                  **inputs)
    return out
